# revision 14
# baseline (speedup 1.0000x reference)
"""Trainium2 Bass kernel for nn_PermLayer: out = x @ permKey.

permKey is a 512x512 one-hot permutation matrix, so the GEMM is exact in
fp32 (each output element is a copy of one input element). Memory-bound:
128 MB in + 128 MB out.

Strategy (data-parallel over batch, 8 cores x 8192 rows):
  per [128, 512] tile of x:
    pass 1: 4x TensorE transpose -> xT chunks [k,b] in PSUM; ACT copies to SBUF
    pass 2: 4x TRANSPOSE-MODE matmuls with the one-hot permKey chunk as the
            moving operand, accumulating in one PSUM bank:
            out += xT_c^T @ P_c. Transpose mode streams fp32 at 2 cycles/row
            (vs 4 for a normal fp32 matmul) and is bit-exact for a 0/1
            moving operand (verified on HW). Accumulation joins must use
            start=False, stop=True - a transpose-mode matmul with stop=False
            hard-hangs the device (verified on HW).
  DMAs batched into 2 MB megatiles; inputs ride the SP HWDGE ring, outputs
  the ACT HWDGE ring so the two streams overlap.
"""

import sys
from contextlib import ExitStack

import numpy as np

sys.path.insert(0, "/opt/trn_rl_repo")

import concourse.bass as bass
import concourse.mybir as mybir
import concourse.tile as tile
from concourse import bacc
from concourse.bass_utils import run_bass_kernel_spmd
from concourse.masks import make_identity

B, N = 65536, 512
NCORES = 8
BS = B // NCORES  # 8192 rows per core
F32 = mybir.dt.float32

# megatile: T sub-tiles of [128, 512] rows loaded per DMA (T*256KB per DMA)
T = 8  # 2 MB per DMA


def build_nc() -> bass.Bass:
    nc = bacc.Bacc("TRN2", target_bir_lowering=False, debug=False)
    x_d = nc.declare_dram_parameter("x", [BS, N], F32, isOutput=False)
    p_d = nc.declare_dram_parameter("permKey", [N, N], F32, isOutput=False)
    o_d = nc.declare_dram_parameter("out", [BS, N], F32, isOutput=True)

    kc = N // 128  # 4 contraction chunks

    with ExitStack() as ctx:
        tc = ctx.enter_context(tile.TileContext(nc))
        const_pool = ctx.enter_context(tc.tile_pool(name="const", bufs=1))
        xpool = ctx.enter_context(tc.tile_pool(name="xin", bufs=3))
        opool = ctx.enter_context(tc.tile_pool(name="oout", bufs=3))
        xtpool = ctx.enter_context(tc.tile_pool(name="xt", bufs=3))
        ps_t = ctx.enter_context(tc.tile_pool(name="ps_t", bufs=2, space="PSUM"))
        ps_o = ctx.enter_context(tc.tile_pool(name="ps_o", bufs=2, space="PSUM"))

        ident = const_pool.tile([128, 128], F32)
        make_identity(nc, ident[:])

        # P chunks: pk[:, c, :] = P[c*128 + p, :]
        pk = const_pool.tile([128, kc, N], F32)
        nc.sync.dma_start(pk[:], p_d.rearrange("(c p) n -> p c n", p=128))

        n_mega = BS // (128 * T)
        for m in range(n_mega):
            xb = xpool.tile([128, T, N], F32)
            nc.sync.dma_start(
                xb[:],
                x_d[m * T * 128 : (m + 1) * T * 128, :].rearrange(
                    "(t p) n -> p t n", p=128
                ),
            )
            ob = opool.tile([128, T, N], F32)
            for t in range(T):
                xt_ps = ps_t.tile([128, N], F32)
                for c in range(kc):
                    nc.tensor.transpose(
                        xt_ps[:, c * 128 : (c + 1) * 128],
                        xb[:, t, c * 128 : (c + 1) * 128],
                        ident[:],
                    )
                xt_sb = xtpool.tile([128, N], F32)
                nc.scalar.copy(xt_sb[:], xt_ps[:])
                out_ps = ps_o.tile([128, N], F32)
                for c in range(kc):
                    # transpose-mode "matmul": exact for one-hot moving
                    # operand; accumulate with start=False, stop=True
                    # (stop=False in transpose mode hangs the device)
                    nc.tensor.matmul(
                        out_ps[:],
                        xt_sb[:, c * 128 : (c + 1) * 128],
                        pk[:, c, :],
                        is_transpose=True,
                        start=(c == 0),
                        stop=True,
                        skip_group_check=(c != 0),
                    )
                nc.vector.tensor_copy(ob[:, t, :], out_ps[:])
            # outputs go out on the ACT HWDGE ring so in/out streams overlap
            nc.scalar.dma_start(
                o_d[m * T * 128 : (m + 1) * T * 128, :].rearrange(
                    "(t p) n -> p t n", p=128
                ),
                ob[:],
            )
    nc.finalize()
    return nc


_NC = None


def _get_nc():
    global _NC
    if _NC is None:
        _NC = build_nc()
    return _NC


def kernel(x: np.ndarray, permKey: np.ndarray) -> np.ndarray:
    x = np.ascontiguousarray(x, dtype=np.float32)
    permKey = np.ascontiguousarray(permKey, dtype=np.float32)
    nc = _get_nc()
    in_maps = [
        {"x": x[i * BS : (i + 1) * BS], "permKey": permKey} for i in range(NCORES)
    ]
    res = run_bass_kernel_spmd(nc, in_maps, list(range(NCORES)))
    return np.concatenate([res.results[i]["out"] for i in range(NCORES)], axis=0)
